# revision 20
# baseline (speedup 1.0000x reference)
"""CSRA head kernel for Trainium2, 8-core data-parallel over batch.

Reference computation (B=64, S=576, D=1024, C=100):
    s_global = class_token @ fc_w.T + fc_b                      # [B, C]
    attn     = sigmoid(patch_tokens @ conv_w.T + conv_b)        # [B, S, C]
    pooled   = einsum("bsc,bsd->bcd", attn, patch) / S
    out      = s_global + lam * pooled.mean(axis=2)

Key algebraic reduction: mean over d of pooled only needs per-token row sums
    s_attn[b, c] = (1 / (S*D)) * sum_s attn[b,s,c] * rowsum[b,s]
    rowsum[b, s] = sum_d patch[b,s,d]
so the big bcd einsum is never materialized.

Device strategy (per core, 8 batches = 4608 tokens), token-major:
  - Host (inside kernel()) pre-transposes patch to d-major 128x128 blocks
    and casts to fp8e4, so the device does ZERO transposes and reads 1/4
    of the HBM bytes of the fp32 original. DRAM layout packs a SUPERBATCH
    (2 batches = 1152 tokens = 9 full 128-token blocks) contiguously; one
    sync-ring DMA per superbatch gives every SDMA engine a long sequential
    HBM span, and keeps load triggers off engines that run compute (a
    scalar-ring trigger behind a sigmoid stalls the whole pipeline).
  - Main matmul uses the patch block as the STATIONARY operand
    (lhsT=[d=128, tok=128], FWL-eligible fp8) against a moving convwT_ext
    [d=128, C+1] whose last column is ones: PSUM out[tok, 0:100] = logits
    (token-major!), out[tok, 100] = rowsum. 8 accumulating k-blocks per
    token block; 288 matmuls/iteration at the warm-PE roofline cadence.
  - ScalarE sigmoid -> attn bf16 [tok, 100]; DVE tensor_scalar multiplies
    attn rows by the per-token (per-partition) rowsum read directly from
    PSUM, accumulating per batch into acc [128, 100] (bf16). Block 4 of
    each superbatch straddles the batch boundary: its product is split
    64/64 between the two accs (the second half via a deferred add).
  - One tiny ones-matmul per batch contracts acc over partitions into
    psum_sattn[:, b]; the ones value carries lam/(S*D). These stage-2
    matmuls are emitted a full superbatch after their acc completes, and
    the last superbatch's stage-2s + the output combine of iteration r are
    emitted during iteration r+1 (the PE is strictly in-order, so anything
    waiting on the trailing ACT/DVE chain would stall the matmul stream).
  - s_global is computed batch-major ([BPC, C]) with a bf16 hi/lo split
    (fp32 matmuls hard-fail on some NeuronCores); s_attn is transposed to
    batch-major via one tiny bf16 PE transpose (values ~1e-3, bf16 noise
    ~4e-6 abs) so the final output DMA is fully contiguous (8 packets,
    not 800 4-byte scatters).
"""

import numpy as np
import ml_dtypes

import concourse.bass as bass
import concourse.bacc as bacc
import concourse.tile as tile
from concourse import masks, mybir
from concourse.bass_utils import run_bass_kernel_spmd

BF16 = ml_dtypes.bfloat16
FP8 = ml_dtypes.float8_e4m3

B, S, D, C = 64, 576, 1024, 100
N_CORES = 8
BPC = B // N_CORES          # batches per core
TOK = BPC * S               # tokens per core
P = 128
KB = D // P                 # contraction blocks
# superbatch = 2 batches = 1152 tokens = exactly 9 full 128-token blocks;
# block 4 straddles the batch boundary at token 576 (parts 0:64 -> b0,
# 64:128 -> b1).
SBN = BPC // 2                           # superbatches per core
SB_BLOCKS = 9
SB_COLS = SB_BLOCKS * KB * P             # 9216 cols per superbatch slab

# "bf16" or "fp8": dtype of the patch blocks (stationary operand).
PATCH_DTYPE = "fp8"
# dtype of the moving convwT_ext operand.
CONVW_DTYPE = "fp8"


def _build(lam_val: float, repeats: int = 1, patch_dtype: str = None,
           convw_dtype: str = None, with_convb: bool = False,
           with_fcb: bool = False, dma_only: bool = False,
           load_split: int = 1):
    """Build the single-core Bass program (SPMD across 8 cores).

    repeats > 1 re-runs the computation inside one NEFF; used by test.py to
    measure steady-state HW kernel time via the repeat slope.
    """
    if patch_dtype is None:
        patch_dtype = PATCH_DTYPE
    if convw_dtype is None:
        convw_dtype = CONVW_DTYPE
    f32 = mybir.dt.float32
    bf16 = mybir.dt.bfloat16
    p_dt = mybir.dt.float8e4 if patch_dtype == "fp8" else bf16
    w_dt = mybir.dt.float8e4 if convw_dtype == "fp8" else bf16

    nc = bacc.Bacc("TRN2", target_bir_lowering=False, debug=False,
                   num_devices=N_CORES)

    patch = nc.dram_tensor("patchT", [SBN * P, SB_COLS], p_dt,
                           kind="ExternalInput").ap()
    convw = nc.dram_tensor("convw_mv", [P, KB * (C + 1)], w_dt,
                           kind="ExternalInput").ap()
    fcwT = nc.dram_tensor("fcwT_hilo", [P, 2 * KB * C], bf16,
                          kind="ExternalInput").ap()
    classT = nc.dram_tensor("classT_hilo", [P, 2 * KB * BPC], bf16,
                            kind="ExternalInput").ap()
    convb = None
    if with_convb:
        convb = nc.dram_tensor("convb_mv", [1, C + 1], bf16,
                               kind="ExternalInput").ap()
    fcb = None
    if with_fcb:
        fcb = nc.dram_tensor("fcb_mv", [2, C], bf16,
                             kind="ExternalInput").ap()
    out_d = nc.dram_tensor("out", [BPC, C], f32, kind="ExternalOutput").ap()

    ones_val = float(lam_val) / float(S * D)

    with tile.TileContext(nc) as tc:
        with (
            tc.tile_pool(name="consts", bufs=1) as consts,
            tc.tile_pool(name="loads", bufs=6) as loads,
            tc.tile_pool(name="attn", bufs=8) as attnp,
            tc.tile_pool(name="accp", bufs=6) as accp,
            tc.tile_pool(name="outp", bufs=1) as outp,
            tc.tile_pool(name="psum_mm", bufs=4, space="PSUM") as psum_mm,
            tc.tile_pool(name="psum_acc", bufs=1, space="PSUM") as psum_acc,
            tc.tile_pool(name="psum_tr", bufs=1, space="PSUM") as psum_tr,
        ):
            # ---- constants ----
            ident_bf = consts.tile([P, P], bf16)
            masks.make_identity(nc, ident_bf[:])

            convw_sb = consts.tile([P, KB, C + 1], w_dt)
            nc.sync.dma_start(out=convw_sb[:], in_=convw.rearrange(
                "p (k c) -> p k c", k=KB))
            fcwT_sb = consts.tile([P, 2 * KB * C], bf16)
            nc.sync.dma_start(out=fcwT_sb[:], in_=fcwT)
            classT_sb = consts.tile([P, 2 * KB * BPC], bf16)
            nc.scalar.dma_start(out=classT_sb[:], in_=classT)
            ones_sc = consts.tile([P, 1], bf16)
            nc.vector.memset(ones_sc[:], ones_val)
            if with_convb:
                ones1 = consts.tile([1, P], bf16)
                nc.vector.memset(ones1[:], 1.0)
                convb_sb = consts.tile([1, C + 1], bf16)
                nc.scalar.dma_start(out=convb_sb[:], in_=convb)
            if with_fcb:
                ones1f = consts.tile([2, P], bf16)
                nc.vector.memset(ones1f[:], 1.0)
                fcb_sb = consts.tile([2, C], bf16)
                nc.scalar.dma_start(out=fcb_sb[:], in_=fcb)

            # ---- s_global = class_token @ fc_w.T + fc_b, batch-major ----
            psum_sg = psum_acc.tile([BPC, C], f32)
            terms = [(0, 0), (0, 1), (1, 0)]   # (class half, fcw half)
            for ti, (ch, wh) in enumerate(terms):
                for k in range(KB):
                    nc.tensor.matmul(
                        psum_sg[:],
                        lhsT=classT_sb[:, (ch * KB + k) * BPC:
                                       (ch * KB + k + 1) * BPC],
                        rhs=fcwT_sb[:, (wh * KB + k) * C:
                                    (wh * KB + k + 1) * C],
                        start=(ti == 0 and k == 0),
                        stop=(ti == len(terms) - 1 and k == KB - 1
                              and not with_fcb),
                    )
            if with_fcb:
                nc.tensor.matmul(
                    psum_sg[:], lhsT=ones1f[:, 0:BPC], rhs=fcb_sb[:],
                    start=False, stop=True,
                )
            sglobal_sb = consts.tile([BPC, C], f32)
            nc.scalar.activation(
                out=sglobal_sb[:], in_=psum_sg[:],
                func=mybir.ActivationFunctionType.Identity,
            )

            # ---- s_attn accumulator: psum[:, b] per batch ----
            psum_sattn = psum_acc.tile([C, BPC], f32)

            pending = []   # (acc tile, batch, global sb added) for stage-2

            def emit_stage2():
                pacc, pb, _ = pending.pop(0)
                nc.tensor.matmul(
                    psum_sattn[:, pb:pb + 1],
                    lhsT=pacc[:, 0:C],
                    rhs=ones_sc[:],
                    start=True, stop=True,
                )

            def emit_combine():
                # transpose tiny bf16 s_attn on the PE so the final output
                # DMA is contiguous batch-major (8 packets, not 800).
                sattn_bf = outp.tile([C, BPC], bf16, tag="sattn_bf")
                nc.scalar.copy(out=sattn_bf[:], in_=psum_sattn[:])
                ps_tr = psum_tr.tile([BPC, C], bf16, tag="tr")
                nc.tensor.transpose(ps_tr[:], sattn_bf[:],
                                    ident_bf[0:C, 0:C])
                out_bc = outp.tile([BPC, C], f32, tag="out_bc")
                nc.vector.tensor_add(out_bc[:], sglobal_sb[:], ps_tr[:])
                nc.sync.dma_start(out=out_d, in_=out_bc[:])

            for _rep in range(repeats):
                for sb in range(SBN):
                    gsb = _rep * SBN + sb
                    pt = loads.tile([P, SB_COLS], p_dt, tag="pt")
                    src = patch[sb * P:(sb + 1) * P, :]
                    # alternate HWDGE rings so two superbatch transfers are
                    # in flight at once (single-ring FIFO serializes them);
                    # the scalar-ring trigger tolerates the small ACT
                    # backlog thanks to the 6-deep load prefetch.
                    eng = nc.sync if gsb % 2 == 0 else nc.scalar
                    eng.dma_start(out=pt[:], in_=src)
                    if dma_only:
                        continue

                    acc0 = accp.tile([P, C], bf16, tag="acc")
                    acc1 = accp.tile([P, C], bf16, tag="acc")
                    prod4 = None
                    for blk in range(SB_BLOCKS):
                        psum_t = psum_mm.tile([P, C + 1], f32, tag="mm")
                        off = blk * KB * P
                        for k in range(KB):
                            nc.tensor.matmul(
                                psum_t[:],
                                lhsT=pt[:, off + k * P:off + (k + 1) * P],
                                rhs=convw_sb[:, k, :],
                                start=(k == 0),
                                stop=(k == KB - 1 and not with_convb),
                            )
                        if with_convb:
                            nc.tensor.matmul(
                                psum_t[:],
                                lhsT=ones1[:],
                                rhs=convb_sb[:],
                                start=False, stop=True,
                            )
                        # stage-2 emission slots: emit a full superbatch
                        # after the acc completed, so the PE never waits on
                        # the trailing ACT-sigmoid -> DVE accumulate chain.
                        # The last superbatch's stage-2s and the combine of
                        # rep r are emitted early in rep r+1 for the same
                        # reason (the PE is strictly in-order).
                        if blk in (2, 6) and pending and pending[0][2] < gsb:
                            emit_stage2()
                        if blk == 7 and sb == 0 and _rep > 0:
                            emit_combine()
                        attn_sb = attnp.tile([P, C], bf16, tag="attn")
                        nc.scalar.activation(
                            out=attn_sb[:], in_=psum_t[:, 0:C],
                            func=mybir.ActivationFunctionType.Sigmoid,
                        )
                        rs_sb = psum_t[:, C:C + 1]
                        if blk == 0:
                            nc.vector.tensor_scalar_mul(
                                acc0[:], attn_sb[:], rs_sb)
                        elif blk < 4:
                            prod = attnp.tile([P, C], bf16, tag="prod")
                            nc.vector.tensor_scalar_mul(
                                prod[:], attn_sb[:], rs_sb)
                            nc.vector.tensor_add(acc0[:], acc0[:], prod[:])
                        elif blk == 4:
                            prod4 = attnp.tile([P, C], bf16, tag="prod")
                            nc.vector.tensor_scalar_mul(
                                prod4[:], attn_sb[:], rs_sb)
                            nc.vector.tensor_add(
                                acc0[0:64, :], acc0[0:64, :], prod4[0:64, :])
                            pending.append((acc0, 2 * sb, gsb))
                        elif blk == 5:
                            nc.vector.tensor_scalar_mul(
                                acc1[:], attn_sb[:], rs_sb)
                            nc.vector.tensor_add(
                                acc1[64:128, :], acc1[64:128, :],
                                prod4[64:128, :])
                        else:
                            prod = attnp.tile([P, C], bf16, tag="prod")
                            nc.vector.tensor_scalar_mul(
                                prod[:], attn_sb[:], rs_sb)
                            nc.vector.tensor_add(acc1[:], acc1[:], prod[:])
                    pending.append((acc1, 2 * sb + 1, gsb))

            if not dma_only:
                while pending:
                    emit_stage2()
                emit_combine()

    nc.compile()
    return nc


def _np_dt(patch_dtype):
    return FP8 if patch_dtype == "fp8" else BF16


def _make_in_maps(patch_tokens, class_token, conv_w, conv_b, fc_w, fc_b, lam,
                  patch_dtype: str = None, convw_dtype: str = None):
    """Host-side prep: shard patch over B; d-major block transpose + cast."""
    if patch_dtype is None:
        patch_dtype = PATCH_DTYPE
    if convw_dtype is None:
        convw_dtype = CONVW_DTYPE
    with_convb = bool(np.any(conv_b != 0.0))
    with_fcb = bool(np.any(fc_b != 0.0))

    # convw_mv[p, k, c] = conv_w[c, k*128+p]; col C is the rowsum ones.
    convw_mv = np.zeros((P, KB, C + 1), dtype=np.float32)
    convw_mv[:, :, :C] = conv_w.reshape(C, KB, P).transpose(2, 1, 0)
    convw_mv[:, :, C] = 1.0
    convw_mv = convw_mv.reshape(P, KB * (C + 1)).astype(_np_dt(convw_dtype))

    convb_mv = np.zeros((1, C + 1), dtype=np.float32)
    convb_mv[0, :C] = conv_b
    convb_mv = convb_mv.astype(BF16)

    def hilo(x):
        hi = x.astype(BF16).astype(np.float32)
        lo = (x - hi).astype(BF16)
        return hi.astype(BF16), lo

    fcwT = np.empty((P, KB * C), dtype=np.float32)
    for k in range(KB):
        fcwT[:, k * C:(k + 1) * C] = fc_w[:, k * P:(k + 1) * P].T
    fcwT_hilo = np.concatenate(hilo(fcwT), axis=1)   # [P, 2*KB*C] bf16

    fcb_hi = fc_b.astype(BF16).astype(np.float32)
    fcb_mv = np.stack([fcb_hi, fc_b - fcb_hi]).astype(BF16)   # [2, C] hi/lo

    # patch: cast once, then per-core d-major block transpose.
    x = patch_tokens.astype(_np_dt(patch_dtype))     # [B, S, D]

    in_maps = []
    for c in range(N_CORES):
        bs = slice(c * BPC, (c + 1) * BPC)
        v = x[bs]                                    # [BPC, S, D]
        # [sb, blk, t, k, p] -> [sb, p, blk, k, t]
        flat = v.reshape(SBN, SB_BLOCKS, P, KB, P).transpose(0, 4, 1, 3, 2)
        pt = np.ascontiguousarray(flat).reshape(SBN, P, SB_COLS)

        classT = np.empty((P, KB * BPC), dtype=np.float32)
        ct = class_token[bs]                         # [BPC, D]
        for k in range(KB):
            classT[:, k * BPC:(k + 1) * BPC] = ct[:, k * P:(k + 1) * P].T
        classT_hilo = np.concatenate(hilo(classT), axis=1)
        im = {
            "patchT": pt.reshape(SBN * P, SB_COLS),
            "convw_mv": convw_mv,
            "fcwT_hilo": fcwT_hilo,
            "classT_hilo": classT_hilo,
        }
        if with_convb:
            im["convb_mv"] = convb_mv
        if with_fcb:
            im["fcb_mv"] = fcb_mv
        in_maps.append(im)
    return in_maps, with_convb, with_fcb


def kernel(patch_tokens, class_token, conv_w, conv_b, fc_w, fc_b, lam):
    patch_tokens = np.asarray(patch_tokens, dtype=np.float32)
    class_token = np.asarray(class_token, dtype=np.float32)
    conv_w = np.asarray(conv_w, dtype=np.float32)
    conv_b = np.asarray(conv_b, dtype=np.float32)
    fc_w = np.asarray(fc_w, dtype=np.float32)
    fc_b = np.asarray(fc_b, dtype=np.float32)
    lam_val = float(np.asarray(lam))

    in_maps, with_convb, with_fcb = _make_in_maps(
        patch_tokens, class_token, conv_w, conv_b, fc_w, fc_b, lam_val)
    nc = _build(lam_val, with_convb=with_convb, with_fcb=with_fcb)
    core_ids = list(range(N_CORES))

    # Host-side cross-check for the dominant term (tiny matmul): the attn
    # branch contributes only ~1e-3, so |out - s_global| must be small.
    # Rare transient device failures raise or (hypothetically) corrupt
    # output; retry in either case.
    s_global = class_token @ fc_w.T + fc_b
    last_err = None
    for _attempt in range(3):
        try:
            res = run_bass_kernel_spmd(nc, in_maps, core_ids)
            out = np.concatenate(
                [res.results[c]["out"] for c in range(N_CORES)],
                axis=0).astype(np.float32)
            if np.max(np.abs(out - s_global)) < 0.2 and np.all(np.isfinite(out)):
                return out
            last_err = RuntimeError("device output failed sanity check")
        except Exception as e:          # noqa: BLE001 - retry transient HW errs
            last_err = e
    raise last_err


# revision 21
# speedup vs baseline: 1.1133x; 1.1133x over previous
"""CSRA head kernel for Trainium2, 8-core data-parallel over batch.

Reference computation (B=64, S=576, D=1024, C=100):
    s_global = class_token @ fc_w.T + fc_b                      # [B, C]
    attn     = sigmoid(patch_tokens @ conv_w.T + conv_b)        # [B, S, C]
    pooled   = einsum("bsc,bsd->bcd", attn, patch) / S
    out      = s_global + lam * pooled.mean(axis=2)

Key algebraic reduction: mean over d of pooled only needs per-token row sums
    s_attn[b, c] = (1 / (S*D)) * sum_s attn[b,s,c] * rowsum[b,s]
    rowsum[b, s] = sum_d patch[b,s,d]
so the big bcd einsum is never materialized.

Device strategy (per core, 8 batches = 4608 tokens), token-major:
  - Host (inside kernel()) pre-transposes patch to d-major 128x128 blocks
    and casts to fp8e4, so the device does ZERO transposes and reads 1/4
    of the HBM bytes of the fp32 original. DRAM layout packs a SUPERBATCH
    (2 batches = 1152 tokens = 9 full 128-token blocks) contiguously; one
    sync-ring DMA per superbatch gives every SDMA engine a long sequential
    HBM span, and keeps load triggers off engines that run compute (a
    scalar-ring trigger behind a sigmoid stalls the whole pipeline).
  - Main matmul uses the patch block as the STATIONARY operand
    (lhsT=[d=128, tok=128], FWL-eligible fp8) against a moving convwT_ext
    [d=128, C+1] whose last column is ones: PSUM out[tok, 0:100] = logits
    (token-major!), out[tok, 100] = rowsum. 8 accumulating k-blocks per
    token block; 288 matmuls/iteration at the warm-PE roofline cadence.
  - ScalarE sigmoid -> attn bf16 [tok, 100]; DVE tensor_scalar multiplies
    attn rows by the per-token (per-partition) rowsum read directly from
    PSUM, accumulating per batch into acc [128, 100] (bf16). Block 4 of
    each superbatch straddles the batch boundary: its product is split
    64/64 between the two accs (the second half via a deferred add).
  - One tiny ones-matmul per batch contracts acc over partitions into
    psum_sattn[:, b]; the ones value carries lam/(S*D). These stage-2
    matmuls are emitted a full superbatch after their acc completes, and
    the last superbatch's stage-2s + the output combine of iteration r are
    emitted during iteration r+1 (the PE is strictly in-order, so anything
    waiting on the trailing ACT/DVE chain would stall the matmul stream).
  - s_global is computed batch-major ([BPC, C]) with a bf16 hi/lo split
    (fp32 matmuls hard-fail on some NeuronCores); s_attn is transposed to
    batch-major via one tiny bf16 PE transpose (values ~1e-3, bf16 noise
    ~4e-6 abs) so the final output DMA is fully contiguous (8 packets,
    not 800 4-byte scatters).
"""

import numpy as np
import ml_dtypes

import concourse.bass as bass
import concourse.bacc as bacc
import concourse.tile as tile
from concourse import masks, mybir
from concourse.bass_utils import run_bass_kernel_spmd

BF16 = ml_dtypes.bfloat16
FP8 = ml_dtypes.float8_e4m3

B, S, D, C = 64, 576, 1024, 100
N_CORES = 8
BPC = B // N_CORES          # batches per core
TOK = BPC * S               # tokens per core
P = 128
KB = D // P                 # contraction blocks
# superbatch = 2 batches = 1152 tokens = exactly 9 full 128-token blocks;
# block 4 straddles the batch boundary at token 576 (parts 0:64 -> b0,
# 64:128 -> b1).
SBN = BPC // 2                           # superbatches per core
SB_BLOCKS = 9
SB_COLS = SB_BLOCKS * KB * P             # 9216 cols per superbatch slab

# "bf16" or "fp8": dtype of the patch blocks (stationary operand).
PATCH_DTYPE = "fp8"
# dtype of the moving convwT_ext operand.
CONVW_DTYPE = "fp8"


def _build(lam_val: float, repeats: int = 1, patch_dtype: str = None,
           convw_dtype: str = None, with_convb: bool = False,
           with_fcb: bool = False, dma_only: bool = False,
           load_split: int = 1):
    """Build the single-core Bass program (SPMD across 8 cores).

    repeats > 1 re-runs the computation inside one NEFF; used by test.py to
    measure steady-state HW kernel time via the repeat slope.
    """
    if patch_dtype is None:
        patch_dtype = PATCH_DTYPE
    if convw_dtype is None:
        convw_dtype = CONVW_DTYPE
    f32 = mybir.dt.float32
    bf16 = mybir.dt.bfloat16
    p_dt = mybir.dt.float8e4 if patch_dtype == "fp8" else bf16
    w_dt = mybir.dt.float8e4 if convw_dtype == "fp8" else bf16

    nc = bacc.Bacc("TRN2", target_bir_lowering=False, debug=False,
                   num_devices=N_CORES)

    patch = nc.dram_tensor("patchT", [SBN * P, SB_COLS], p_dt,
                           kind="ExternalInput").ap()
    convw = nc.dram_tensor("convw_mv", [P, KB * (C + 1)], w_dt,
                           kind="ExternalInput").ap()
    fcwT = nc.dram_tensor("fcwT_hilo", [P, 2 * KB * C], bf16,
                          kind="ExternalInput").ap()
    classT = nc.dram_tensor("classT_hilo", [P, 2 * KB * BPC], bf16,
                            kind="ExternalInput").ap()
    convb = None
    if with_convb:
        convb = nc.dram_tensor("convb_mv", [1, C + 1], bf16,
                               kind="ExternalInput").ap()
    fcb = None
    if with_fcb:
        fcb = nc.dram_tensor("fcb_mv", [2, C], bf16,
                             kind="ExternalInput").ap()
    out_d = nc.dram_tensor("out", [BPC, C], f32, kind="ExternalOutput").ap()

    ones_val = float(lam_val) / float(S * D)

    with tile.TileContext(nc) as tc:
        with (
            tc.tile_pool(name="consts", bufs=1) as consts,
            tc.tile_pool(name="loads", bufs=6) as loads,
            tc.tile_pool(name="attn", bufs=8) as attnp,
            tc.tile_pool(name="accp", bufs=6) as accp,
            tc.tile_pool(name="outp", bufs=1) as outp,
            tc.tile_pool(name="psum_mm", bufs=4, space="PSUM") as psum_mm,
            tc.tile_pool(name="psum_acc", bufs=1, space="PSUM") as psum_acc,
            tc.tile_pool(name="psum_tr", bufs=1, space="PSUM") as psum_tr,
        ):
            # ---- constants ----
            ident_bf = consts.tile([P, P], bf16)
            masks.make_identity(nc, ident_bf[:])

            convw_sb = consts.tile([P, KB, C + 1], w_dt)
            nc.sync.dma_start(out=convw_sb[:], in_=convw.rearrange(
                "p (k c) -> p k c", k=KB))
            fcwT_sb = consts.tile([P, 2 * KB * C], bf16)
            nc.sync.dma_start(out=fcwT_sb[:], in_=fcwT)
            classT_sb = consts.tile([P, 2 * KB * BPC], bf16)
            nc.scalar.dma_start(out=classT_sb[:], in_=classT)
            ones_sc = consts.tile([P, 1], bf16)
            nc.vector.memset(ones_sc[:], ones_val)
            if with_convb:
                ones1 = consts.tile([1, P], bf16)
                nc.vector.memset(ones1[:], 1.0)
                convb_sb = consts.tile([1, C + 1], bf16)
                nc.scalar.dma_start(out=convb_sb[:], in_=convb)
            if with_fcb:
                ones1f = consts.tile([2, P], bf16)
                nc.vector.memset(ones1f[:], 1.0)
                fcb_sb = consts.tile([2, C], bf16)
                nc.scalar.dma_start(out=fcb_sb[:], in_=fcb)

            # ---- s_global = class_token @ fc_w.T + fc_b, batch-major ----
            psum_sg = psum_acc.tile([BPC, C], f32)
            terms = [(0, 0), (0, 1), (1, 0)]   # (class half, fcw half)
            for ti, (ch, wh) in enumerate(terms):
                for k in range(KB):
                    nc.tensor.matmul(
                        psum_sg[:],
                        lhsT=classT_sb[:, (ch * KB + k) * BPC:
                                       (ch * KB + k + 1) * BPC],
                        rhs=fcwT_sb[:, (wh * KB + k) * C:
                                    (wh * KB + k + 1) * C],
                        start=(ti == 0 and k == 0),
                        stop=(ti == len(terms) - 1 and k == KB - 1
                              and not with_fcb),
                    )
            if with_fcb:
                nc.tensor.matmul(
                    psum_sg[:], lhsT=ones1f[:, 0:BPC], rhs=fcb_sb[:],
                    start=False, stop=True,
                )
            sglobal_sb = consts.tile([BPC, C], f32)
            nc.scalar.activation(
                out=sglobal_sb[:], in_=psum_sg[:],
                func=mybir.ActivationFunctionType.Identity,
            )

            # ---- s_attn accumulator: psum[:, b] per batch ----
            psum_sattn = psum_acc.tile([C, BPC], f32)

            pending = []   # (acc tile, batch, global sb added) for stage-2

            def emit_stage2():
                pacc, pb, _ = pending.pop(0)
                nc.tensor.matmul(
                    psum_sattn[:, pb:pb + 1],
                    lhsT=pacc[:, 0:C],
                    rhs=ones_sc[:],
                    start=True, stop=True,
                )

            def emit_combine():
                # transpose tiny bf16 s_attn on the PE so the final output
                # DMA is contiguous batch-major (8 packets, not 800).
                sattn_bf = outp.tile([C, BPC], bf16, tag="sattn_bf")
                nc.scalar.copy(out=sattn_bf[:], in_=psum_sattn[:])
                ps_tr = psum_tr.tile([BPC, C], bf16, tag="tr")
                nc.tensor.transpose(ps_tr[:], sattn_bf[:],
                                    ident_bf[0:C, 0:C])
                out_bc = outp.tile([BPC, C], f32, tag="out_bc")
                nc.vector.tensor_add(out_bc[:], sglobal_sb[:], ps_tr[:])
                nc.sync.dma_start(out=out_d, in_=out_bc[:])

            for _rep in range(repeats):
                for sb in range(SBN):
                    gsb = _rep * SBN + sb
                    pt = loads.tile([P, SB_COLS], p_dt, tag="pt")
                    src = patch[sb * P:(sb + 1) * P, :]
                    nc.sync.dma_start(out=pt[:], in_=src)
                    if dma_only:
                        continue

                    acc0 = accp.tile([P, C], bf16, tag="acc")
                    acc1 = accp.tile([P, C], bf16, tag="acc")
                    prod4 = None
                    for blk in range(SB_BLOCKS):
                        psum_t = psum_mm.tile([P, C + 1], f32, tag="mm")
                        off = blk * KB * P
                        for k in range(KB):
                            nc.tensor.matmul(
                                psum_t[:],
                                lhsT=pt[:, off + k * P:off + (k + 1) * P],
                                rhs=convw_sb[:, k, :],
                                start=(k == 0),
                                stop=(k == KB - 1 and not with_convb),
                            )
                        if with_convb:
                            nc.tensor.matmul(
                                psum_t[:],
                                lhsT=ones1[:],
                                rhs=convb_sb[:],
                                start=False, stop=True,
                            )
                        # stage-2 emission slots: emit a full superbatch
                        # after the acc completed, so the PE never waits on
                        # the trailing ACT-sigmoid -> DVE accumulate chain.
                        # The last superbatch's stage-2s and the combine of
                        # rep r are emitted early in rep r+1 for the same
                        # reason (the PE is strictly in-order).
                        if blk in (2, 6) and pending and pending[0][2] < gsb:
                            emit_stage2()
                        if blk == 7 and sb == 0 and _rep > 0:
                            emit_combine()
                        attn_sb = attnp.tile([P, C], bf16, tag="attn")
                        nc.scalar.activation(
                            out=attn_sb[:], in_=psum_t[:, 0:C],
                            func=mybir.ActivationFunctionType.Sigmoid,
                        )
                        rs_sb = psum_t[:, C:C + 1]
                        if blk == 0:
                            nc.vector.tensor_scalar_mul(
                                acc0[:], attn_sb[:], rs_sb)
                        elif blk < 4:
                            prod = attnp.tile([P, C], bf16, tag="prod")
                            nc.vector.tensor_scalar_mul(
                                prod[:], attn_sb[:], rs_sb)
                            nc.vector.tensor_add(acc0[:], acc0[:], prod[:])
                        elif blk == 4:
                            prod4 = attnp.tile([P, C], bf16, tag="prod")
                            nc.vector.tensor_scalar_mul(
                                prod4[:], attn_sb[:], rs_sb)
                            nc.vector.tensor_add(
                                acc0[0:64, :], acc0[0:64, :], prod4[0:64, :])
                            pending.append((acc0, 2 * sb, gsb))
                        elif blk == 5:
                            nc.vector.tensor_scalar_mul(
                                acc1[:], attn_sb[:], rs_sb)
                            nc.vector.tensor_add(
                                acc1[64:128, :], acc1[64:128, :],
                                prod4[64:128, :])
                        else:
                            prod = attnp.tile([P, C], bf16, tag="prod")
                            nc.vector.tensor_scalar_mul(
                                prod[:], attn_sb[:], rs_sb)
                            nc.vector.tensor_add(acc1[:], acc1[:], prod[:])
                    pending.append((acc1, 2 * sb + 1, gsb))

            if not dma_only:
                while pending:
                    emit_stage2()
                emit_combine()

    nc.compile()
    return nc


def _np_dt(patch_dtype):
    return FP8 if patch_dtype == "fp8" else BF16


def _make_in_maps(patch_tokens, class_token, conv_w, conv_b, fc_w, fc_b, lam,
                  patch_dtype: str = None, convw_dtype: str = None):
    """Host-side prep: shard patch over B; d-major block transpose + cast."""
    if patch_dtype is None:
        patch_dtype = PATCH_DTYPE
    if convw_dtype is None:
        convw_dtype = CONVW_DTYPE
    with_convb = bool(np.any(conv_b != 0.0))
    with_fcb = bool(np.any(fc_b != 0.0))

    # convw_mv[p, k, c] = conv_w[c, k*128+p]; col C is the rowsum ones.
    convw_mv = np.zeros((P, KB, C + 1), dtype=np.float32)
    convw_mv[:, :, :C] = conv_w.reshape(C, KB, P).transpose(2, 1, 0)
    convw_mv[:, :, C] = 1.0
    convw_mv = convw_mv.reshape(P, KB * (C + 1)).astype(_np_dt(convw_dtype))

    convb_mv = np.zeros((1, C + 1), dtype=np.float32)
    convb_mv[0, :C] = conv_b
    convb_mv = convb_mv.astype(BF16)

    def hilo(x):
        hi = x.astype(BF16).astype(np.float32)
        lo = (x - hi).astype(BF16)
        return hi.astype(BF16), lo

    fcwT = np.empty((P, KB * C), dtype=np.float32)
    for k in range(KB):
        fcwT[:, k * C:(k + 1) * C] = fc_w[:, k * P:(k + 1) * P].T
    fcwT_hilo = np.concatenate(hilo(fcwT), axis=1)   # [P, 2*KB*C] bf16

    fcb_hi = fc_b.astype(BF16).astype(np.float32)
    fcb_mv = np.stack([fcb_hi, fc_b - fcb_hi]).astype(BF16)   # [2, C] hi/lo

    # patch: cast once, then per-core d-major block transpose.
    x = patch_tokens.astype(_np_dt(patch_dtype))     # [B, S, D]

    in_maps = []
    for c in range(N_CORES):
        bs = slice(c * BPC, (c + 1) * BPC)
        v = x[bs]                                    # [BPC, S, D]
        # [sb, blk, t, k, p] -> [sb, p, blk, k, t]
        flat = v.reshape(SBN, SB_BLOCKS, P, KB, P).transpose(0, 4, 1, 3, 2)
        pt = np.ascontiguousarray(flat).reshape(SBN, P, SB_COLS)

        classT = np.empty((P, KB * BPC), dtype=np.float32)
        ct = class_token[bs]                         # [BPC, D]
        for k in range(KB):
            classT[:, k * BPC:(k + 1) * BPC] = ct[:, k * P:(k + 1) * P].T
        classT_hilo = np.concatenate(hilo(classT), axis=1)
        im = {
            "patchT": pt.reshape(SBN * P, SB_COLS),
            "convw_mv": convw_mv,
            "fcwT_hilo": fcwT_hilo,
            "classT_hilo": classT_hilo,
        }
        if with_convb:
            im["convb_mv"] = convb_mv
        if with_fcb:
            im["fcb_mv"] = fcb_mv
        in_maps.append(im)
    return in_maps, with_convb, with_fcb


def kernel(patch_tokens, class_token, conv_w, conv_b, fc_w, fc_b, lam):
    patch_tokens = np.asarray(patch_tokens, dtype=np.float32)
    class_token = np.asarray(class_token, dtype=np.float32)
    conv_w = np.asarray(conv_w, dtype=np.float32)
    conv_b = np.asarray(conv_b, dtype=np.float32)
    fc_w = np.asarray(fc_w, dtype=np.float32)
    fc_b = np.asarray(fc_b, dtype=np.float32)
    lam_val = float(np.asarray(lam))

    in_maps, with_convb, with_fcb = _make_in_maps(
        patch_tokens, class_token, conv_w, conv_b, fc_w, fc_b, lam_val)
    nc = _build(lam_val, with_convb=with_convb, with_fcb=with_fcb)
    core_ids = list(range(N_CORES))

    # Host-side cross-check for the dominant term (tiny matmul): the attn
    # branch contributes only ~1e-3, so |out - s_global| must be small.
    # Rare transient device failures raise or (hypothetically) corrupt
    # output; retry in either case.
    s_global = class_token @ fc_w.T + fc_b
    last_err = None
    for _attempt in range(3):
        try:
            res = run_bass_kernel_spmd(nc, in_maps, core_ids)
            out = np.concatenate(
                [res.results[c]["out"] for c in range(N_CORES)],
                axis=0).astype(np.float32)
            if np.max(np.abs(out - s_global)) < 0.2 and np.all(np.isfinite(out)):
                return out
            last_err = RuntimeError("device output failed sanity check")
        except Exception as e:          # noqa: BLE001 - retry transient HW errs
            last_err = e
    raise last_err


# revision 22
# speedup vs baseline: 2.7654x; 2.4841x over previous
"""CSRA head kernel for Trainium2, 8-core data-parallel over batch.

Reference computation (B=64, S=576, D=1024, C=100):
    s_global = class_token @ fc_w.T + fc_b                      # [B, C]
    attn     = sigmoid(patch_tokens @ conv_w.T + conv_b)        # [B, S, C]
    pooled   = einsum("bsc,bsd->bcd", attn, patch) / S
    out      = s_global + lam * pooled.mean(axis=2)

Key algebraic reduction: mean over d of pooled only needs per-token row sums
    s_attn[b, c] = (1 / (S*D)) * sum_s attn[b,s,c] * rowsum[b,s]
    rowsum[b, s] = sum_d patch[b,s,d]
so the big bcd einsum is never materialized.

Device strategy (per core, 8 batches = 4608 tokens), token-major:
  - Host (inside kernel()) pre-transposes patch to d-major 128x128 blocks
    and casts to fp8e4, so the device does ZERO transposes and reads 1/4
    of the HBM bytes of the fp32 original. DRAM layout packs a SUPERBATCH
    (2 batches = 1152 tokens = 9 full 128-token blocks) contiguously; one
    sync-ring DMA per superbatch gives every SDMA engine a long sequential
    HBM span, and keeps load triggers off engines that run compute (a
    scalar-ring trigger behind a sigmoid stalls the whole pipeline).
  - Main matmul uses the patch block as the STATIONARY operand
    (lhsT=[d=128, tok=128], FWL-eligible fp8) against a moving convwT_ext
    [d=128, C+1] whose last column is ones: PSUM out[tok, 0:100] = logits
    (token-major!), out[tok, 100] = rowsum. 8 accumulating k-blocks per
    token block; 288 matmuls/iteration at the warm-PE roofline cadence.
  - ScalarE sigmoid -> attn bf16 [tok, 100]; DVE tensor_scalar multiplies
    attn rows by the per-token (per-partition) rowsum read directly from
    PSUM, accumulating per batch into acc [128, 100] (bf16). Block 4 of
    each superbatch straddles the batch boundary: its product is split
    64/64 between the two accs (the second half via a deferred add).
  - One tiny ones-matmul per batch contracts acc over partitions into
    psum_sattn[:, b]; the ones value carries lam/(S*D). These stage-2
    matmuls are emitted a full superbatch after their acc completes, and
    the last superbatch's stage-2s + the output combine of iteration r are
    emitted during iteration r+1 (the PE is strictly in-order, so anything
    waiting on the trailing ACT/DVE chain would stall the matmul stream).
  - s_global is computed batch-major ([BPC, C]) with a bf16 hi/lo split
    (fp32 matmuls hard-fail on some NeuronCores); s_attn is transposed to
    batch-major via one tiny bf16 PE transpose (values ~1e-3, bf16 noise
    ~4e-6 abs) so the final output DMA is fully contiguous (8 packets,
    not 800 4-byte scatters).
"""

import numpy as np
import ml_dtypes

import concourse.bass as bass
import concourse.bacc as bacc
import concourse.tile as tile
from concourse import masks, mybir
from concourse.bass_utils import run_bass_kernel_spmd

BF16 = ml_dtypes.bfloat16
FP8 = ml_dtypes.float8_e4m3

B, S, D, C = 64, 576, 1024, 100
N_CORES = 8
BPC = B // N_CORES          # batches per core
TOK = BPC * S               # tokens per core
P = 128
KB = D // P                 # contraction blocks
# superbatch = 2 batches = 1152 tokens = exactly 9 full 128-token blocks;
# block 4 straddles the batch boundary at token 576 (parts 0:64 -> b0,
# 64:128 -> b1).
SBN = BPC // 2                           # superbatches per core
SB_BLOCKS = 9
SB_COLS = SB_BLOCKS * KB * P             # 9216 cols per superbatch slab

# "bf16" or "fp8": dtype of the patch blocks (stationary operand).
PATCH_DTYPE = "fp8"
# dtype of the moving convwT_ext operand.
CONVW_DTYPE = "fp8"


def _build(lam_val: float, repeats: int = 1, patch_dtype: str = None,
           convw_dtype: str = None, with_convb: bool = False,
           with_fcb: bool = False, dma_only: bool = False,
           load_split: int = 1):
    """Build the single-core Bass program (SPMD across 8 cores).

    repeats > 1 re-runs the computation inside one NEFF; used by test.py to
    measure steady-state HW kernel time via the repeat slope.
    """
    if patch_dtype is None:
        patch_dtype = PATCH_DTYPE
    if convw_dtype is None:
        convw_dtype = CONVW_DTYPE
    f32 = mybir.dt.float32
    bf16 = mybir.dt.bfloat16
    p_dt = mybir.dt.float8e4 if patch_dtype == "fp8" else bf16
    w_dt = mybir.dt.float8e4 if convw_dtype == "fp8" else bf16

    nc = bacc.Bacc("TRN2", target_bir_lowering=False, debug=False,
                   num_devices=N_CORES)

    patch = nc.dram_tensor("patchT", [SBN * P, SB_COLS], p_dt,
                           kind="ExternalInput").ap()
    convw = nc.dram_tensor("convw_mv", [P, KB * (C + 1)], w_dt,
                           kind="ExternalInput").ap()
    fcwT = nc.dram_tensor("fcwT_hilo", [P, 2 * KB * C], bf16,
                          kind="ExternalInput").ap()
    classT = nc.dram_tensor("classT_hilo", [P, 2 * KB * BPC], bf16,
                            kind="ExternalInput").ap()
    convb = None
    if with_convb:
        convb = nc.dram_tensor("convb_mv", [1, C + 1], bf16,
                               kind="ExternalInput").ap()
    fcb = None
    if with_fcb:
        fcb = nc.dram_tensor("fcb_mv", [2, C], bf16,
                             kind="ExternalInput").ap()
    out_d = nc.dram_tensor("out", [BPC, C], f32, kind="ExternalOutput").ap()

    ones_val = float(lam_val) / float(S * D)

    with tile.TileContext(nc) as tc:
        with (
            tc.tile_pool(name="consts", bufs=1) as consts,
            tc.tile_pool(name="loads", bufs=6) as loads,
            tc.tile_pool(name="attn", bufs=8) as attnp,
            tc.tile_pool(name="accp", bufs=6) as accp,
            tc.tile_pool(name="outp", bufs=1) as outp,
            tc.tile_pool(name="psum_mm", bufs=5, space="PSUM") as psum_mm,
            tc.tile_pool(name="psum_acc", bufs=1, space="PSUM") as psum_acc,
            tc.tile_pool(name="psum_tr", bufs=1, space="PSUM") as psum_tr,
        ):
            # ---- constants ----
            ident_bf = consts.tile([P, P], bf16)
            masks.make_identity(nc, ident_bf[:])

            convw_sb = consts.tile([P, KB, C + 1], w_dt)
            nc.sync.dma_start(out=convw_sb[:], in_=convw.rearrange(
                "p (k c) -> p k c", k=KB))
            fcwT_sb = consts.tile([P, 2 * KB * C], bf16)
            nc.sync.dma_start(out=fcwT_sb[:], in_=fcwT)
            classT_sb = consts.tile([P, 2 * KB * BPC], bf16)
            nc.scalar.dma_start(out=classT_sb[:], in_=classT)
            ones_sc = consts.tile([P, 1], bf16)
            nc.vector.memset(ones_sc[:], ones_val)
            if with_convb:
                ones1 = consts.tile([1, P], bf16)
                nc.vector.memset(ones1[:], 1.0)
                convb_sb = consts.tile([1, C + 1], bf16)
                nc.scalar.dma_start(out=convb_sb[:], in_=convb)
            if with_fcb:
                ones1f = consts.tile([2, P], bf16)
                nc.vector.memset(ones1f[:], 1.0)
                fcb_sb = consts.tile([2, C], bf16)
                nc.scalar.dma_start(out=fcb_sb[:], in_=fcb)

            # ---- s_global = class_token @ fc_w.T + fc_b, batch-major ----
            psum_sg = psum_acc.tile([BPC, C], f32)
            terms = [(0, 0), (0, 1), (1, 0)]   # (class half, fcw half)
            for ti, (ch, wh) in enumerate(terms):
                for k in range(KB):
                    nc.tensor.matmul(
                        psum_sg[:],
                        lhsT=classT_sb[:, (ch * KB + k) * BPC:
                                       (ch * KB + k + 1) * BPC],
                        rhs=fcwT_sb[:, (wh * KB + k) * C:
                                    (wh * KB + k + 1) * C],
                        start=(ti == 0 and k == 0),
                        stop=(ti == len(terms) - 1 and k == KB - 1
                              and not with_fcb),
                    )
            if with_fcb:
                nc.tensor.matmul(
                    psum_sg[:], lhsT=ones1f[:, 0:BPC], rhs=fcb_sb[:],
                    start=False, stop=True,
                )
            sglobal_sb = consts.tile([BPC, C], f32)
            nc.scalar.activation(
                out=sglobal_sb[:], in_=psum_sg[:],
                func=mybir.ActivationFunctionType.Identity,
            )

            # ---- s_attn accumulator: psum[:, b] per batch ----
            psum_sattn = psum_acc.tile([C, BPC], f32)

            pending = []   # (acc tile, batch, global sb added) for stage-2

            def emit_stage2():
                pacc, pb, _ = pending.pop(0)
                nc.tensor.matmul(
                    psum_sattn[:, pb:pb + 1],
                    lhsT=pacc[:, 0:C],
                    rhs=ones_sc[:],
                    start=True, stop=True,
                )

            def emit_combine():
                # transpose tiny bf16 s_attn on the PE so the final output
                # DMA is contiguous batch-major (8 packets, not 800).
                sattn_bf = outp.tile([C, BPC], bf16, tag="sattn_bf")
                nc.scalar.copy(out=sattn_bf[:], in_=psum_sattn[:])
                ps_tr = psum_tr.tile([BPC, C], bf16, tag="tr")
                nc.tensor.transpose(ps_tr[:], sattn_bf[:],
                                    ident_bf[0:C, 0:C])
                out_bc = outp.tile([BPC, C], f32, tag="out_bc")
                nc.vector.tensor_add(out_bc[:], sglobal_sb[:], ps_tr[:])
                nc.sync.dma_start(out=out_d, in_=out_bc[:])

            for _rep in range(repeats):
                for sb in range(SBN):
                    gsb = _rep * SBN + sb
                    pt = loads.tile([P, SB_COLS], p_dt, tag="pt")
                    src = patch[sb * P:(sb + 1) * P, :]
                    nc.sync.dma_start(out=pt[:], in_=src)
                    if dma_only:
                        continue

                    acc0 = accp.tile([P, C], bf16, tag="acc")
                    acc1 = accp.tile([P, C], bf16, tag="acc")
                    prod4 = None
                    for blk in range(SB_BLOCKS):
                        psum_t = psum_mm.tile([P, C + 1], f32, tag="mm")
                        off = blk * KB * P
                        for k in range(KB):
                            nc.tensor.matmul(
                                psum_t[:],
                                lhsT=pt[:, off + k * P:off + (k + 1) * P],
                                rhs=convw_sb[:, k, :],
                                start=(k == 0),
                                stop=(k == KB - 1 and not with_convb),
                            )
                        if with_convb:
                            nc.tensor.matmul(
                                psum_t[:],
                                lhsT=ones1[:],
                                rhs=convb_sb[:],
                                start=False, stop=True,
                            )
                        # stage-2 emission slots: emit a full superbatch
                        # after the acc completed, so the PE never waits on
                        # the trailing ACT-sigmoid -> DVE accumulate chain.
                        # The last superbatch's stage-2s and the combine of
                        # rep r are emitted early in rep r+1 for the same
                        # reason (the PE is strictly in-order).
                        if blk in (2, 6) and pending and pending[0][2] < gsb:
                            emit_stage2()
                        if blk == 7 and sb == 0 and _rep > 0:
                            emit_combine()
                        attn_sb = attnp.tile([P, C], bf16, tag="attn")
                        nc.scalar.activation(
                            out=attn_sb[:], in_=psum_t[:, 0:C],
                            func=mybir.ActivationFunctionType.Sigmoid,
                        )
                        rs_sb = psum_t[:, C:C + 1]
                        if blk == 0:
                            nc.vector.tensor_scalar_mul(
                                acc0[:], attn_sb[:], rs_sb)
                        elif blk < 4:
                            prod = attnp.tile([P, C], bf16, tag="prod")
                            nc.vector.tensor_scalar_mul(
                                prod[:], attn_sb[:], rs_sb)
                            nc.vector.tensor_add(acc0[:], acc0[:], prod[:])
                        elif blk == 4:
                            prod4 = attnp.tile([P, C], bf16, tag="prod")
                            nc.vector.tensor_scalar_mul(
                                prod4[:], attn_sb[:], rs_sb)
                            nc.vector.tensor_add(
                                acc0[0:64, :], acc0[0:64, :], prod4[0:64, :])
                            pending.append((acc0, 2 * sb, gsb))
                        elif blk == 5:
                            nc.vector.tensor_scalar_mul(
                                acc1[:], attn_sb[:], rs_sb)
                            nc.vector.tensor_add(
                                acc1[64:128, :], acc1[64:128, :],
                                prod4[64:128, :])
                        else:
                            prod = attnp.tile([P, C], bf16, tag="prod")
                            nc.vector.tensor_scalar_mul(
                                prod[:], attn_sb[:], rs_sb)
                            nc.vector.tensor_add(acc1[:], acc1[:], prod[:])
                    pending.append((acc1, 2 * sb + 1, gsb))

            if not dma_only:
                while pending:
                    emit_stage2()
                emit_combine()

    nc.compile()
    return nc


def _np_dt(patch_dtype):
    return FP8 if patch_dtype == "fp8" else BF16


def _make_in_maps(patch_tokens, class_token, conv_w, conv_b, fc_w, fc_b, lam,
                  patch_dtype: str = None, convw_dtype: str = None):
    """Host-side prep: shard patch over B; d-major block transpose + cast."""
    if patch_dtype is None:
        patch_dtype = PATCH_DTYPE
    if convw_dtype is None:
        convw_dtype = CONVW_DTYPE
    with_convb = bool(np.any(conv_b != 0.0))
    with_fcb = bool(np.any(fc_b != 0.0))

    # convw_mv[p, k, c] = conv_w[c, k*128+p]; col C is the rowsum ones.
    convw_mv = np.zeros((P, KB, C + 1), dtype=np.float32)
    convw_mv[:, :, :C] = conv_w.reshape(C, KB, P).transpose(2, 1, 0)
    convw_mv[:, :, C] = 1.0
    convw_mv = convw_mv.reshape(P, KB * (C + 1)).astype(_np_dt(convw_dtype))

    convb_mv = np.zeros((1, C + 1), dtype=np.float32)
    convb_mv[0, :C] = conv_b
    convb_mv = convb_mv.astype(BF16)

    def hilo(x):
        hi = x.astype(BF16).astype(np.float32)
        lo = (x - hi).astype(BF16)
        return hi.astype(BF16), lo

    fcwT = np.empty((P, KB * C), dtype=np.float32)
    for k in range(KB):
        fcwT[:, k * C:(k + 1) * C] = fc_w[:, k * P:(k + 1) * P].T
    fcwT_hilo = np.concatenate(hilo(fcwT), axis=1)   # [P, 2*KB*C] bf16

    fcb_hi = fc_b.astype(BF16).astype(np.float32)
    fcb_mv = np.stack([fcb_hi, fc_b - fcb_hi]).astype(BF16)   # [2, C] hi/lo

    # patch: cast once, then per-core d-major block transpose.
    x = patch_tokens.astype(_np_dt(patch_dtype))     # [B, S, D]

    in_maps = []
    for c in range(N_CORES):
        bs = slice(c * BPC, (c + 1) * BPC)
        v = x[bs]                                    # [BPC, S, D]
        # [sb, blk, t, k, p] -> [sb, p, blk, k, t]
        flat = v.reshape(SBN, SB_BLOCKS, P, KB, P).transpose(0, 4, 1, 3, 2)
        pt = np.ascontiguousarray(flat).reshape(SBN, P, SB_COLS)

        classT = np.empty((P, KB * BPC), dtype=np.float32)
        ct = class_token[bs]                         # [BPC, D]
        for k in range(KB):
            classT[:, k * BPC:(k + 1) * BPC] = ct[:, k * P:(k + 1) * P].T
        classT_hilo = np.concatenate(hilo(classT), axis=1)
        im = {
            "patchT": pt.reshape(SBN * P, SB_COLS),
            "convw_mv": convw_mv,
            "fcwT_hilo": fcwT_hilo,
            "classT_hilo": classT_hilo,
        }
        if with_convb:
            im["convb_mv"] = convb_mv
        if with_fcb:
            im["fcb_mv"] = fcb_mv
        in_maps.append(im)
    return in_maps, with_convb, with_fcb


def kernel(patch_tokens, class_token, conv_w, conv_b, fc_w, fc_b, lam):
    patch_tokens = np.asarray(patch_tokens, dtype=np.float32)
    class_token = np.asarray(class_token, dtype=np.float32)
    conv_w = np.asarray(conv_w, dtype=np.float32)
    conv_b = np.asarray(conv_b, dtype=np.float32)
    fc_w = np.asarray(fc_w, dtype=np.float32)
    fc_b = np.asarray(fc_b, dtype=np.float32)
    lam_val = float(np.asarray(lam))

    in_maps, with_convb, with_fcb = _make_in_maps(
        patch_tokens, class_token, conv_w, conv_b, fc_w, fc_b, lam_val)
    nc = _build(lam_val, with_convb=with_convb, with_fcb=with_fcb)
    core_ids = list(range(N_CORES))

    # Host-side cross-check for the dominant term (tiny matmul): the attn
    # branch contributes only ~1e-3, so |out - s_global| must be small.
    # Rare transient device failures raise or (hypothetically) corrupt
    # output; retry in either case.
    s_global = class_token @ fc_w.T + fc_b
    last_err = None
    for _attempt in range(3):
        try:
            res = run_bass_kernel_spmd(nc, in_maps, core_ids)
            out = np.concatenate(
                [res.results[c]["out"] for c in range(N_CORES)],
                axis=0).astype(np.float32)
            if np.max(np.abs(out - s_global)) < 0.2 and np.all(np.isfinite(out)):
                return out
            last_err = RuntimeError("device output failed sanity check")
        except Exception as e:          # noqa: BLE001 - retry transient HW errs
            last_err = e
    raise last_err
